# revision 1
# baseline (speedup 1.0000x reference)
"""ConvVMamba TRN2 Bass kernel (v2).

Sharding: data-parallel over batch. B=8 -> one image per NeuronCore, all
weights replicated, no collectives.

Per-core layout: channels on SBUF partitions (C=96), pixels on the free dim
(L=64*64=4096).
  - 1x1 convs / projections: PE matmuls (lhsT = W^T, rhs = activations).
  - depthwise 7x7 / 3x3: PE accumulating matmuls with per-tap diagonal lhsT
    over a zero-padded image buffer, TAP-OUTER over 8 PSUM banks so the
    per-tap LDWEIGHTS is emitted once per tap (consecutive same-weight
    matmuls get their ldweights deduped) instead of once per matmul.
  - LayerNorm over channels: partition reduce+broadcast via ones-matmul,
    rstd via a single ACT Rsqrt, squares on GPSIMD.
  - selective scan (d_state=1, A=-1): dA = sigmoid(-z) directly (one ACT),
    delta = -ln(dA) (sign folded into the B selector), DVE/GPSIMD
    tensor_tensor_scan, reversed directions via negative-step APs,
    transposed directions by keeping the per-direction pipeline w-major.

ACT function-table discipline: the bacc table-load pass greedily loads the
first act_info set containing the required function, so each scheduler
phase uses functions that live in one set (Rsqrt -> set 14, Gelu -> 10,
Silu -> 18, Sigmoid -> 2, Ln -> 5; Identity/Copy/Square live in every
set). ACT ops are chained in emission order so phases stay contiguous:
~10 table loads total instead of 131.
"""

import sys
import numpy as np

sys.path.insert(0, "/opt/trn_rl_repo")

import ml_dtypes  # noqa: E402
import concourse.bass as bass  # noqa: E402
import concourse.bacc as bacc  # noqa: E402
import concourse.mybir as mybir  # noqa: E402
from concourse import tile  # noqa: E402
from concourse.tile import add_dep_helper  # noqa: E402
from concourse.bass_utils import run_bass_kernel_spmd  # noqa: E402

F32 = mybir.dt.float32
F32R = mybir.dt.float32r
BF16 = mybir.dt.bfloat16
AF = mybir.ActivationFunctionType
OP = mybir.AluOpType
bfnp = ml_dtypes.bfloat16

B, C, H, W = 8, 96, 64, 64
L = H * W
R, N, K = 6, 1, 4
EPS = 1e-5
P7, P3 = 70, 66  # padded widths for 7x7 and 3x3 convs
NCHUNK = 8  # 4096 / 512
CH = 512

# which scan directions run on GPSIMD (rest on DVE).
# NOTE: InstTensorScalarPtr (the scan opcode) is not in any GPSIMD library,
# so Pool scans fail walrus codegen — keep empty.
POOL_SCAN_DIRS = ()

_CACHE = {}


def _taps(k):
    r = (k - 1) // 2
    return [(dh, dw) for dh in range(k) for dw in range(k)], r


def build_host_tensors(kw):
    """Precompute all weight/constant DRAM tensors (shared across cores)."""
    f = lambda a: np.asarray(a, np.float32)
    out = {}

    # --- fold LN gamma/beta into following 1x1 convs ---
    def fold(wname, bname, g, b):
        w = f(kw[wname])
        bb = f(kw[bname])
        return w * f(g)[None, :], bb + w @ f(b)

    fc1w, fc1b = fold("cn_fc1_w", "cn_fc1_b", kw["cn_ln_w"], kw["cn_ln_b"])
    ipw, ipb = fold("ip_w", "ip_b", kw["v_ln1_w"], kw["v_ln1_b"])
    opw, opb = fold("op_w", "op_b", kw["o_ln_w"], kw["o_ln_b"])
    mfc1w, mfc1b = fold("m_fc1_w", "m_fc1_b", kw["v_ln2_w"], kw["v_ln2_b"])
    fc2w, fc2b = f(kw["cn_fc2_w"]), f(kw["cn_fc2_b"])
    mfc2w, mfc2b = f(kw["m_fc2_w"]), f(kw["m_fc2_b"])

    # --- depthwise conv diagonals, 32x32 block form: [96, 58*32] ---
    # Channel group g = c//32 holds its own 32x32 diagonal mini-lhsT so each
    # tap runs as 3 concurrent tile_position=(32g,32g) matmuls (distinct
    # row/col groups -> minis and their ldweights overlap on the PE).
    w7 = f(kw["cn_dw_w"]).reshape(C, 49)
    w3 = f(kw["dw_w"]).reshape(C, 9)
    diag = np.zeros((C, 58 * 32), np.float32)
    ar = np.arange(C)
    for t in range(49):
        diag[ar, t * 32 + (ar % 32)] = w7[:, t]
    for t in range(9):
        diag[ar, (49 + t) * 32 + (ar % 32)] = w3[:, t]
    out["wdiag"] = diag.astype(bfnp)

    # --- GEMM weights (lhsT layouts), bf16 ---
    out["wfc1"] = fc1w.T.astype(bfnp)  # [96, 384]
    wfc2 = np.zeros((128, 3 * C), np.float32)  # [128, 288] K-chunks
    for j in range(3):
        wfc2[:, j * C:(j + 1) * C] = fc2w[:, j * 128:(j + 1) * 128].T
    out["wfc2"] = wfc2.astype(bfnp)
    out["wip"] = ipw.T.astype(bfnp)  # [96, 96]
    out["wop"] = opw.T.astype(bfnp)
    out["wmfc1"] = mfc1w.T.astype(bfnp)
    wm2 = np.zeros((128, 3 * C), np.float32)
    for j in range(3):
        wm2[:, j * C:(j + 1) * C] = mfc2w[:, j * 128:(j + 1) * 128].T
    out["wmfc2"] = wm2.astype(bfnp)

    # Composed per-direction projections, all [96,96] lhsT from v4 directly
    # (v4 = silu(x), no 0.5 fold; the x_proj -> xdbl intermediate is gone):
    #   wz:  z'_k = -(dt_w_k @ xp_dt_k) @ u_k - dt_b  (sigmoid(z') = dA)
    #   wnB: rank-1 broadcast of -B_k over 96 partitions
    #        (bso = lndelta*u*(-B) = delta*u*B since lndelta = -delta)
    #   wC:  rank-1 broadcast of C_k
    xp = f(kw["x_proj_w"])  # [4, 8, 96]: rows 0:6 dt, 6 B, 7 C
    dtw = f(kw["dt_w"])  # [4, 96, 6]
    wz = np.zeros((C, 4 * C), np.float32)
    wnB = np.zeros((C, 4 * C), np.float32)
    wC = np.zeros((C, 4 * C), np.float32)
    for k in range(4):
        m = dtw[k] @ xp[k][:R]          # [96(d), 96(c)]
        wz[:, k * C:(k + 1) * C] = -m.T
        wnB[:, k * C:(k + 1) * C] = -xp[k][R][:, None]
        wC[:, k * C:(k + 1) * C] = xp[k][R + 1][:, None]
    out["wz"] = wz.astype(bfnp)
    out["wnB"] = wnB.astype(bfnp)
    out["wC"] = wC.astype(bfnp)

    # ones for LN partition reduce+broadcast
    out["ones96"] = np.ones((C, C), np.float32)
    out["ones96_bf"] = np.ones((C, C), bfnp)

    # per-partition scalar bank [128, NV] fp32
    A = (-np.exp(f(kw["A_logs"]))).reshape(K, C)
    a_is_neg1 = bool(np.allclose(A, -1.0, atol=1e-6))
    Ds = f(kw["Ds"]).reshape(K, C)
    dtb = f(kw["dt_b"])  # [4, 96]
    cols = []

    def col(v, n=C):
        a = np.zeros(128, np.float32)
        a[: len(v)] = v
        cols.append(a)
        return len(cols) - 1

    ix = {}
    ix["cn_dw_b"] = col(f(kw["cn_dw_b"]))
    for j in range(3):
        ix[f"fc1b{j}"] = col(fc1b[j * 128:(j + 1) * 128])
    ix["fc2b"] = col(fc2b)
    ix["ipb"] = col(ipb)
    ix["dwb"] = col(f(kw["dw_b"]))
    for k in range(4):
        ix[f"ndtb{k}"] = col(-dtb[k])   # sigmoid bias: z' = -(z + dtb)
        ix[f"nA{k}"] = col(-A[k])       # general-A fallback: dA=exp(-A*lnd)
    ix["Dsum"] = col(Ds.sum(0))
    ix["eps"] = col(np.full(128, EPS, np.float32), 128)
    ix["opb"] = col(opb)
    for j in range(3):
        ix[f"mfc1b{j}"] = col(mfc1b[j * 128:(j + 1) * 128])
    ix["mfc2b"] = col(mfc2b)
    out["vecs"] = np.stack(cols, axis=1)  # [128, NV]
    return out, ix, a_is_neg1


def pad_image(x):
    """[96,64,64] fp32 -> padded [96,70*70] bf16."""
    xp = np.zeros((C, P7, P7), np.float32)
    xp[:, 3:3 + H, 3:3 + W] = x
    return xp.reshape(C, P7 * P7).astype(bfnp)


def r32(ap):
    return ap.bitcast(F32R)


def build_program(ix, a_is_neg1=True):
    nc = bacc.Bacc("TRN2", target_bir_lowering=False, debug=False)

    din = {}
    for name, shape, dt in [
        ("xpad", [C, P7 * P7], BF16),
        ("xres", [C, L], F32),
        ("wdiag", [C, 58 * 32], BF16),
        ("wfc1", [C, 384], BF16),
        ("wfc2", [128, 3 * C], BF16),
        ("wip", [C, C], BF16),
        ("wop", [C, C], BF16),
        ("wmfc1", [C, 384], BF16),
        ("wmfc2", [128, 3 * C], BF16),
        ("wz", [C, 4 * C], BF16),
        ("wnB", [C, 4 * C], BF16),
        ("wC", [C, 4 * C], BF16),
        ("ones96", [C, C], F32R),
        ("ones96_bf", [C, C], BF16),
        ("vecs", [128, len(ix)], F32),
    ]:
        din[name] = nc.dram_tensor(name, shape, dt, kind="ExternalInput").ap()
    dout = nc.dram_tensor("out", [C, L], F32, kind="ExternalOutput").ap()

    class ActPhase:
        # Chain ALL ACT ops (and explicit table loads) in emission order so
        # the scheduler keeps phases contiguous. Edges touching a table
        # load use sync=False: the load is a pseudo-instruction NRT
        # rewrites, and semaphore waits/updates attached to it are dropped
        # at runtime (device hang). Same-engine order-only edges are
        # enough to pin it.
        def __init__(self):
            self.cur_last = None
            self.last_is_load = False

        def tag(self, bi, is_load=False):
            inst = bi.ins
            if self.cur_last is not None:
                add_dep_helper(inst, self.cur_last,
                               sync=not (is_load or self.last_is_load),
                               reason="act table-set phase fence")
            self.cur_last = inst
            self.last_is_load = is_load
            return bi

    ph = ActPhase()

    with tile.TileContext(nc) as tc:
        from contextlib import ExitStack

        with ExitStack() as ctx:
            const = ctx.enter_context(tc.tile_pool(name="const", bufs=1))
            bigp = ctx.enter_context(tc.tile_pool(name="big", bufs=1))
            sgp = ctx.enter_context(tc.tile_pool(name="sg", bufs=1))
            scanp = ctx.enter_context(tc.tile_pool(name="scan", bufs=3))
            hcp = ctx.enter_context(tc.tile_pool(name="hc", bufs=2))
            accp = ctx.enter_context(tc.tile_pool(name="acc", bufs=2))
            chk = ctx.enter_context(tc.tile_pool(name="chk", bufs=2))
            ps = ctx.enter_context(tc.tile_pool(name="ps", bufs=1, space="PSUM"))

            # ---- PSUM: one pool, 8 bank-sized tiles, hand-rotated ----
            class PsumRot:
                def __init__(self):
                    self.i = 0

                def take(self, parts=128, tagno=None):
                    if tagno is None:
                        tagno = self.i
                        self.i = (self.i + 1) % 8
                    t = ps.tile([128, CH], F32, tag=f"p{tagno}")
                    return t[0:parts, :] if parts != 128 else t[:]

            pr = PsumRot()

            # ---- load constants ----
            cc = {}
            for name, ap in din.items():
                if name in ("xpad", "xres"):
                    continue
                t = const.tile(list(ap.shape), ap.dtype, tag=name)
                nc.sync.dma_start(t[:], ap)
                cc[name] = t
            # Route the bias bank through an ACT copy: the ACT instruction
            # encoding has a single sync-wait slot, so later ACT ops must not
            # need a DMA wait on top of their PSUM wait.
            nv = len(ix)
            vecs_sb = const.tile([128, nv], F32, tag="vecs_sb")
            _ld0 = mybir.InstLoadActFuncSet(
                name=nc.get_next_instruction_name(), ins=[], outs=[])
            _ld0.act_func_set_id = 0
            ph.tag(nc.scalar.add_instruction(_ld0), is_load=True)
            ph.tag(nc.scalar.activation(vecs_sb[:], cc["vecs"][:], AF.Copy))
            # dummy reader absorbs the same-engine RAW wait on vecs_sb so
            # later ACT ops keep a single wait slot for their PSUM input
            scr = const.tile([128, 1], F32, tag="scr")
            ph.tag(nc.scalar.activation(scr[:], vecs_sb[:, 0:1], AF.Copy))
            V = lambda key: vecs_sb[:, ix[key]:ix[key] + 1]
            V96 = lambda key: vecs_sb[:C, ix[key]:ix[key] + 1]

            xpad = bigp.tile([C, P7 * P7], BF16, tag="pad")
            nc.sync.dma_start(xpad[:], din["xpad"])
            xres = bigp.tile([C, L], F32, tag="xres")
            nc.sync.dma_start(xres[:], din["xres"])

            # =============== helpers ===============
            def dwconv_all(src_pad, Wp, ktaps, diag_off):
                """Tap-outer depthwise conv into 8 PSUM banks (one/chunk).

                Each tap is 3 concurrent 32x32 diagonal matmuls at
                tile_position=(32g,32g): distinct row/col groups let the
                minis (and their ldweights) overlap on the PE instead of
                serializing a 96-wide ldweights against every matmul."""
                taps, _ = _taps(ktaps)
                nt = len(taps)
                pts = [ps.tile([128, CH], F32, tag=f"p{j}", name=f"cv{j}")
                       for j in range(8)]
                src3 = src_pad.rearrange("c (h w) -> c h w", w=Wp)
                for t, (dh, dw) in enumerate(taps):
                    wcol = cc["wdiag"][:, (diag_off + t) * 32:
                                       (diag_off + t + 1) * 32]
                    for j in range(NCHUNK):
                        r0 = j * 8
                        rhs = src3[:, r0 + dh:r0 + dh + 8, dw:dw + W]
                        for g in range(3):
                            nc.tensor.matmul(
                                pts[j][32 * g:32 * (g + 1), :],
                                wcol[32 * g:32 * (g + 1), :],
                                rhs[32 * g:32 * (g + 1)],
                                start=(t == 0), stop=(t == nt - 1),
                                skip_group_check=True,
                                tile_position=(32 * g, 32 * g),
                            )
                return [p[0:C, :] for p in pts]

            def load_set(set_id):
                """Pre-place an ACT table load so following interleaved
                functions from that set need no greedy per-op loads.

                Ordering edges use sync=False: the load is a pseudo-
                instruction NRT rewrites, and semaphore waits attached to
                it are dropped at runtime (hangs). Same-engine order-only
                edges keep it in its phase slot without semaphores."""
                ld = mybir.InstLoadActFuncSet(
                    name=nc.get_next_instruction_name(), ins=[], outs=[])
                ld.act_func_set_id = set_id
                return ph.tag(nc.scalar.add_instruction(ld), is_load=True)

            def ln_block(src_tile, src_f32, out_tile):
                """out = (x - mean_c) / sqrt(var_c + eps), chunk-pipelined.

                Per chunk: partition reduce+broadcast via all-ones [96,96]
                lhsT matmuls, d via DVE, square on GPSIMD, then
                rstd = exp(-0.5*ln(v+eps)) with BOTH Ln and Exp resident
                (explicit set-6 load) so chunks flow without a barrier."""
                load_set(6)
                for j in range(NCHUNK):
                    src_chunk = src_tile[:, j * CH:(j + 1) * CH]
                    mb = pr.take(parts=C)
                    if src_f32:
                        nc.tensor.matmul(mb, cc["ones96"][:], r32(src_chunk),
                                         start=True, stop=True)
                    else:
                        nc.tensor.matmul(mb, cc["ones96_bf"][:], src_chunk,
                                         start=True, stop=True)
                    d = chk.tile([C, CH], BF16, tag="lnd")
                    nc.vector.scalar_tensor_tensor(d[:], mb, -1.0 / C,
                                                   src_chunk, OP.mult, OP.add)
                    dsq = chk.tile([C, CH], BF16, tag="sq")
                    nc.gpsimd.tensor_tensor(dsq[:], d[:], d[:], OP.mult)
                    vb = pr.take(parts=C)
                    nc.tensor.matmul(vb, cc["ones96_bf"][:], dsq[:],
                                     start=True, stop=True)
                    lnv = chk.tile([C, CH], BF16, tag="lnv")
                    ph.tag(nc.scalar.activation(lnv[:], vb, AF.Ln,
                                                scale=1.0 / C,
                                                bias=V96("eps")))
                    rstd = chk.tile([C, CH], BF16, tag="rstd")
                    ph.tag(nc.scalar.activation(rstd[:], lnv[:], AF.Exp,
                                                scale=-0.5))
                    nc.vector.tensor_tensor(out_tile[:, j * CH:(j + 1) * CH],
                                            d[:], rstd[:], OP.mult)

            def mlp_block(src_tile, src_f32, wf1, wf2, b1pfx, b2key, res_tile,
                          out_tile, round_out=False):
                """out = res + fc2(gelu(fc1(LN(src)))) ; all chunked."""
                xnf = scanp.tile([C, L], BF16, tag="sc", name="xnf")
                ln_block(src_tile, src_f32, xnf[:])
                load_set(10)
                for j in range(NCHUNK):
                    xn = xnf[:, j * CH:(j + 1) * CH]
                    gs = []
                    for mm in range(3):
                        f1 = pr.take()
                        nc.tensor.matmul(f1, cc[wf1][:, mm * 128:(mm + 1) * 128],
                                         xn, start=True, stop=True)
                        g = chk.tile([128, CH], BF16, tag=f"g{mm}")
                        ph.tag(nc.scalar.activation(g[:], f1, AF.Gelu,
                                                    bias=V(f"{b1pfx}{mm}")))
                        gs.append(g)
                    f2 = pr.take(parts=C)
                    for mm in range(3):
                        nc.tensor.matmul(f2, cc[wf2][:, mm * C:(mm + 1) * C],
                                         gs[mm][:], start=(mm == 0), stop=(mm == 2))
                    oap = out_tile[:, j * CH:(j + 1) * CH]
                    if round_out:
                        oap = oap.bitcast(F32R)
                    nc.vector.scalar_tensor_tensor(
                        oap, f2, V96(b2key),
                        res_tile[:, j * CH:(j + 1) * CH], OP.add, OP.add)

            # =============== ConvNeXt block ===============
            hsb = bigp.tile([C, L], BF16, tag="bufA")
            pts7 = dwconv_all(xpad[:], P7, 7, 0)
            for j in range(NCHUNK):
                ph.tag(nc.scalar.activation(hsb[:, j * CH:(j + 1) * CH],
                                            pts7[j], AF.Identity,
                                            bias=V96("cn_dw_b")))
            x1 = bigp.tile([C, L], F32, tag="x1")
            mlp_block(hsb, False, "wfc1", "wfc2", "fc1b", "fc2b", xres, x1,
                      round_out=True)

            # =============== SS2D: LN1 + in_proj + dwconv3 + silu ==========
            # reuse the (dead) 7x7 pad buffer for the 3x3 padded image
            v2pad_full = bigp.tile([C, P7 * P7], BF16, tag="pad")
            v2pad = v2pad_full[:, 0:P3 * P3]
            nc.gpsimd.memset(v2pad, 0.0)
            v2int = v2pad.rearrange("c (h w) -> c h w", w=P3)
            xn1f = scanp.tile([C, L], BF16, tag="sc", name="xn1f")
            ln_block(x1, True, xn1f[:])
            for j in range(NCHUNK):
                pv = pr.take(parts=C)
                nc.tensor.matmul(pv, cc["wip"][:],
                                 xn1f[:, j * CH:(j + 1) * CH], start=True,
                                 stop=True)
                dst = v2int[:, 1 + j * 8:1 + (j + 1) * 8, 1:1 + W]
                ph.tag(nc.scalar.activation(dst, pv, AF.Identity,
                                            bias=V96("ipb")))
            v4 = bigp.tile([C, L], BF16, tag="bufA")
            pts3 = dwconv_all(v2pad, P3, 3, 49)
            load_set(18)
            for j in range(NCHUNK):
                ph.tag(nc.scalar.activation(v4[:, j * CH:(j + 1) * CH],
                                            pts3[j], AF.Silu,
                                            bias=V96("dwb")))

            # =============== per-direction scan ===============
            # All per-direction tensors (z', -B, C) come from composed
            # [96,96] matmuls reading v4 (l-major) / v4T (w-major) directly.
            # Directions processed in pairs (0,2) then (1,3) so the ACT
            # sigmoid phase (set 2) and ln phase (set 5) each load once per
            # pair. k=2,3 accumulate onto k=0,1 results.
            v4T = v4[:].rearrange("c (h w) -> c h w", w=W).transpose([0, 2, 1])
            accs = {}
            sg_tiles = {}

            def urhs(k, j):
                if k in (0, 2):
                    return v4[:, j * CH:(j + 1) * CH]
                return v4T[:, j * 8:(j + 1) * 8, :]

            def wcol(name, k):
                return cc[name][:, k * C:(k + 1) * C]

            for pair in ((0, 2), (1, 3)):
                # --- sigmoid phase: dA = sigmoid(-(dtproj + dtb)) ---
                load_set(2)
                for k in pair:
                    sg = sgp.tile([C, L], BF16, tag=f"sg{pair.index(k)}",
                                  name=f"sg{k}")
                    sg_tiles[k] = sg
                    for j in range(NCHUNK):
                        pd = pr.take(parts=C)
                        nc.tensor.matmul(pd, wcol("wz", k), urhs(k, j),
                                         start=True, stop=True)
                        ph.tag(nc.scalar.activation(
                            sg[:, j * CH:(j + 1) * CH], pd, AF.Sigmoid,
                            bias=V96(f"ndtb{k}")))
                # --- ln phase: lndelta = ln(dA) = -delta ---
                # (own sgp tags: these live across the whole pair, longer
                # than the scanp rotation window)
                lnds = {}
                load_set(6)
                for k in pair:
                    lnd = sgp.tile([C, L], BF16, tag=f"lnd{pair.index(k)}",
                                   name=f"lnd{k}")
                    ph.tag(nc.scalar.activation(lnd[:], sg_tiles[k][:], AF.Ln))
                    lnds[k] = lnd
                # --- per-direction work: full-L DVE ops (one op instead of
                # 8 chunked ones -> fewer semaphores, 2x bf16 mode) ---
                for k in pair:
                    lnd = lnds[k]
                    if a_is_neg1:
                        dA = sg_tiles[k]
                    else:
                        dA = scanp.tile([C, L], BF16, tag="sc",
                                        name=f"dA{k}")
                        ph.tag(nc.scalar.activation(dA[:], lnd[:], AF.Exp,
                                                    scale=V96(f"nA{k}")))
                    bbe = scanp.tile([C, L], BF16, tag="sc", name=f"bbe{k}")
                    for j in range(NCHUNK):
                        bb = pr.take(parts=C)
                        nc.tensor.matmul(bb, wcol("wnB", k), urhs(k, j),
                                         start=True, stop=True)
                        ph.tag(nc.scalar.activation(
                            bbe[:, j * CH:(j + 1) * CH], bb, AF.Copy))
                    du = scanp.tile([C, L], BF16, tag="sc", name=f"du{k}")
                    if k in (0, 2):
                        nc.vector.tensor_tensor(du[:], lnd[:], v4[:], OP.mult)
                    else:
                        nc.vector.tensor_tensor(du[:], lnd[:], v4T, OP.mult)
                    bso = scanp.tile([C, L], BF16, tag="sc", name=f"bso{k}")
                    nc.vector.tensor_tensor(bso[:], du[:], bbe[:], OP.mult)
                    h = scanp.tile([C, L], BF16, tag="sc", name=f"h{k}")
                    if k in (0, 1):
                        nc.vector.tensor_tensor_scan(h[:], dA[:], bso[:],
                                                     0.0, OP.mult, OP.add)
                    else:
                        nc.vector.tensor_tensor_scan(h[:][:, ::-1],
                                                     dA[:][:, ::-1],
                                                     bso[:][:, ::-1], 0.0,
                                                     OP.mult, OP.add)
                    # y_k = h * Cs_b  (+ accumulate onto k-2's result)
                    cbe = scanp.tile([C, L], BF16, tag="sc", name=f"cbe{k}")
                    for j in range(NCHUNK):
                        cb = pr.take(parts=C)
                        nc.tensor.matmul(cb, wcol("wC", k), urhs(k, j),
                                         start=True, stop=True)
                        ph.tag(nc.scalar.activation(
                            cbe[:, j * CH:(j + 1) * CH], cb, AF.Copy))
                    if k in (0, 1):
                        dst = hcp.tile([C, L], BF16, tag="hc", name=f"hc{k}")
                        nc.vector.tensor_tensor(dst[:], h[:], cbe[:], OP.mult)
                    else:
                        dst = accp.tile([C, L], BF16, tag="acc",
                                        name=f"acc{k}")
                        tmp = scanp.tile([C, L], BF16, tag="sc",
                                         name=f"tmp{k}")
                        nc.vector.tensor_tensor(tmp[:], h[:], cbe[:], OP.mult)
                        nc.gpsimd.tensor_tensor(dst[:], accs[k - 2][:],
                                                tmp[:], OP.add)
                    accs[k] = dst

            # =============== cross-merge + D*u + LN + out_proj =============
            preln = hcp.tile([C, L], BF16, tag="hc")
            accT = accs[3][:].rearrange("c (w h) -> c w h", w=W).transpose(
                [0, 2, 1])
            t2 = scanp.tile([C, L], BF16, tag="sc", name="t2")
            nc.vector.tensor_tensor(t2[:], accs[2][:], accT, OP.add)
            nc.vector.scalar_tensor_tensor(preln[:], v4[:], V96("Dsum"),
                                           t2[:], OP.mult, OP.add)
            x2 = bigp.tile([C, L], F32, tag="x2")
            ynf = scanp.tile([C, L], BF16, tag="sc", name="ynf")
            ln_block(preln, False, ynf[:])
            for j in range(NCHUNK):
                po = pr.take(parts=C)
                nc.tensor.matmul(po, cc["wop"][:],
                                 ynf[:, j * CH:(j + 1) * CH], start=True,
                                 stop=True)
                nc.vector.scalar_tensor_tensor(x2[:, j * CH:(j + 1) * CH]
                                               .bitcast(F32R), po,
                                               V96("opb"),
                                               x1[:, j * CH:(j + 1) * CH],
                                               OP.add, OP.add)

            # =============== MLP block ===============
            outsb = bigp.tile([C, L], F32, tag="x1")
            mlp_block(x2, True, "wmfc1", "wmfc2", "mfc1b", "mfc2b", x2, outsb)
            nc.sync.dma_start(dout, outsb[:])

    nc.compile()
    return nc


def get_program_and_inputs(inputs):
    host, ix, a_is_neg1 = build_host_tensors(inputs)
    key = ("prog", a_is_neg1)
    if key not in _CACHE:
        _CACHE[key] = build_program(ix, a_is_neg1)
    nc = _CACHE[key]
    x = np.asarray(inputs["x"], np.float32)
    in_maps = []
    for b in range(B):
        m = {k: v for k, v in host.items()}
        m["xpad"] = pad_image(x[b])
        m["xres"] = x[b].reshape(C, L).astype(np.float32)
        in_maps.append(m)
    return nc, in_maps


def kernel(**inputs):
    nc, in_maps = get_program_and_inputs(inputs)
    res = run_bass_kernel_spmd(nc, in_maps, list(range(B)))
    out = np.stack([res.results[b]["out"].reshape(C, H, W) for b in range(B)])
    return out.astype(np.float32)


if __name__ == "__main__":
    # smoke build
    host, ix, a1 = build_host_tensors(
        {k: np.zeros(s, np.float32) for k, s in [  # noqa

            ("x", (B, C, H, W)), ("cn_dw_w", (C, 7, 7)), ("cn_dw_b", (C,)),
            ("cn_ln_w", (C,)), ("cn_ln_b", (C,)), ("cn_fc1_w", (4 * C, C)),
            ("cn_fc1_b", (4 * C,)), ("cn_fc2_w", (C, 4 * C)), ("cn_fc2_b", (C,)),
            ("v_ln1_w", (C,)), ("v_ln1_b", (C,)), ("ip_w", (C, C)),
            ("ip_b", (C,)), ("dw_w", (C, 3, 3)), ("dw_b", (C,)),
            ("x_proj_w", (K, R + 2 * N, C)), ("dt_w", (K, C, R)),
            ("dt_b", (K, C)), ("A_logs", (K * C, N)), ("Ds", (K * C,)),
            ("o_ln_w", (C,)), ("o_ln_b", (C,)), ("op_w", (C, C)),
            ("op_b", (C,)), ("v_ln2_w", (C,)), ("v_ln2_b", (C,)),
            ("m_fc1_w", (4 * C, C)), ("m_fc1_b", (4 * C,)),
            ("m_fc2_w", (C, 4 * C)), ("m_fc2_b", (C,)),
        ]})
    print("a_is_neg1:", a1)
    nc = build_program(ix, a1)
    print("program built OK:", len(list(nc.all_instructions())), "instructions")

